# Initial kernel scaffold
#
"""BCQLinear (3-bit binary-coded quantized linear) Trainium2 kernel.

Full-input contract: kernel(**inputs) takes the unsharded inputs of
nn_BCQLinear_88510686036218 and returns the full [1, 128, 4096] output.

Math: w = alpha*(2*S-7) + beta with S in [0,8) the 3-bit code, then
y = (x[:, in_reorder] @ w)[:, out_reorder].
Split: device computes y_q = x @ (2*alpha*S); host adds the rank-32
beta part corr = xsum.T @ (beta - 7*alpha) (0.01% of the FLOPs).

Sharding: out-features split 8 ways (512 cols/core), x replicated.

Per-core device program (SPMD, one Bass program):
  - Contraction rows are band-packed: K-tile kt in [0,32), partition p:
    dequant row i(kt,p) = 128*(p//4) + 4*kt + (p%4), so a single [128,512]
    alpha tile (a2p[p,:] ~ alpha[p//4,:]) serves every K-tile.
  - Codes arrive as packed int32 words with 4 3-bit fields per int16
    half (field r at bits [3r,3r+3)); code (j, r) maps to out-col
    o' = r*128 + j.  Dequant is two DVE passes per 8-K-tile chunk:
      unpack: vt = W32 & ((7<<3r) replicated in both halves) -- one
              single-op int32 TensorScalar per r; leaves the code
              scaled by 8^r, which rides through the matmul into z
              and is divided out on the host.
      scale:  wm = vt(int16) * a2p(fp16) -- one full-width TensorTensor
              (the last chunk splits it per column half for the tail).
  - Matmuls: stationary xt[128, T] per K-tile, streaming wm [128, 256]
    halves into two PSUM banks (fp16 operands, fp32 PSUM).
  - All DMAs ride the two HWDGE queues (sync: weight chunks, scalar:
    a2p + xt); keeping the SWDGE/gpsimd queue idle avoids descriptor-
    ring SBUF traffic that measurably slows concurrent DVE ops.
  - The last chunk runs its h1 half first so the two output halves'
    copy + DMA + HBM-receipt chains overlap.
"""
import numpy as np
from contextlib import ExitStack

import concourse.bass as bass
import concourse.mybir as mybir
import concourse.tile as tile
from concourse import bacc

IN_F, OUT_F, WBITS, GS, OFI = 4096, 4096, 3, 128, 128
NG, NB = 32, 32
NCORES = 8
OPC = OUT_F // NCORES        # 512 out-cols per core
NKT = 32                     # K-tiles of 128 rows
NR = 4                       # fields per int16 half
NWC = OPC // (2 * NR)        # 64 packed words per (partition, K-tile)
T = 128                      # tokens
NCHUNK = 4                   # w pipeline chunks
KTC = NKT // NCHUNK          # K-tiles per chunk (8)

F32 = mybir.dt.float32
F16 = mybir.dt.float16
BF16 = mybir.dt.bfloat16
I32 = mybir.dt.int32
I16 = mybir.dt.int16
ALU = mybir.AluOpType

_PROGRAM_CACHE = {}


# ---------------------------------------------------------------- host prep
def _dequant_codes(qweight):
    """S[i, o] in [0,8): w = alpha*(2S-7)+beta."""
    qw = np.asarray(qweight, dtype=np.uint32).reshape(NG, NB, WBITS, GS * OFI // 32)
    bits = (qw[..., None] >> np.arange(32, dtype=np.uint32)) & 1
    bits = bits.reshape(NG, NB, WBITS, GS, OFI)
    S = (bits * (2 ** np.arange(WBITS, dtype=np.uint32))[:, None, None]).sum(axis=2)
    return S.transpose(0, 2, 1, 3).reshape(IN_F, OUT_F).astype(np.uint32)


def _band_rows():
    kt, p = np.meshgrid(np.arange(NKT), np.arange(128), indexing="ij")
    return 128 * (p // 4) + 4 * kt + (p % 4)      # [NKT, 128]


def _prepare(inputs):
    x = np.asarray(inputs["x"], np.float32).reshape(-1, IN_F)
    alpha = np.asarray(inputs["alpha"], np.float32)
    beta = np.asarray(inputs["beta"], np.float32)
    in_reorder = np.asarray(inputs["in_reorder"], np.int64)
    xf = x[:, in_reorder]

    S = _dequant_codes(inputs["qweight"])          # [IN_F, OUT_F] uint32
    rows = _band_rows()                            # [NKT, 128]
    XT = np.ascontiguousarray(
        xf[:, rows.reshape(-1)].T.reshape(NKT, 128, T).transpose(1, 0, 2)
    ).reshape(128, NKT * T).astype(np.float16)     # [p, kt*T]

    # host-side beta part: corr[t, o] = sum_g xsum[g,t] * (beta-7a)[g,o]
    xsum = xf.reshape(T, NG, GS).sum(axis=2, dtype=np.float64)   # [T, NG]
    Bfull = beta.astype(np.float64) - 7.0 * alpha.astype(np.float64)
    corr = (xsum @ Bfull).astype(np.float32)       # [T, OUT_F]

    CW = KTC * NWC                                 # 512 words/chunk/partition
    XQ = NKT * T // NCHUNK
    in_maps = []
    for c in range(NCORES):
        cols = slice(OPC * c, OPC * (c + 1))
        # codes for this core in banded row order: [p, kt, o']
        Sc = S[rows.reshape(-1), cols].reshape(NKT, 128, OPC).transpose(1, 0, 2)
        # pack fields: o' = r*128 + 2c' + h  ->  bits [3r+16h, +3)
        W = np.zeros((128, NKT, NWC), np.uint32)
        for r in range(NR):
            for h in range(2):
                W |= Sc[:, :, r * 128 + h::2][:, :, :NWC] << (3 * r + 16 * h)
        W = W.reshape(128, NKT * NWC)
        a2p = (2.0 * alpha[np.arange(128) // 4][:, cols]).astype(np.float16)
        im = dict(a2p=a2p)
        for ch in range(NCHUNK):
            im[f"w{ch}"] = np.ascontiguousarray(
                W[:, ch * CW:(ch + 1) * CW]).view(np.int32)
        for ch in range(NCHUNK):
            im[f"xt{ch}"] = np.ascontiguousarray(XT[:, ch * XQ:(ch + 1) * XQ])
        in_maps.append(im)
    return in_maps, corr


# ---------------------------------------------------------------- program
def build_program():
    nc = bacc.Bacc("TRN2")
    CW = KTC * NWC            # packed words per chunk per partition (512)
    HW2 = CW // 2             # 256 words per chunk-0 half
    XQ = NKT * T // NCHUNK    # xt columns per chunk (1024)

    w_dr = {ch: nc.declare_dram_parameter(f"w{ch}", [128, CW], I32,
                                          isOutput=False)
            for ch in range(NCHUNK)}
    xt_dr = [nc.declare_dram_parameter(f"xt{ch}", [128, XQ], F16,
                                       isOutput=False)
             for ch in range(NCHUNK)]
    a2p_dr = nc.declare_dram_parameter("a2p", [128, OPC], F16, isOutput=False)
    z = nc.declare_dram_parameter("z", [T, OPC], F16, isOutput=True)

    with tile.TileContext(nc) as tc, ExitStack() as ctx:
        cpool = ctx.enter_context(tc.tile_pool(name="const", bufs=1))
        opool = ctx.enter_context(tc.tile_pool(name="out", bufs=1))
        ppool = ctx.enter_context(tc.tile_pool(name="psum", bufs=1, space="PSUM"))

        # --- static tiles --------------------------------------------------
        w_sb = {ch: cpool.tile([128, CW], I32, tag=f"w{ch}", name=f"wsb{ch}")
                for ch in range(NCHUNK)}
        xt_sb = [cpool.tile([128, XQ], F16, tag=f"xt{ch}", name=f"xtsb{ch}")
                 for ch in range(NCHUNK)]
        a2_sb = cpool.tile([128, OPC], F16, tag="a2")
        # dequantized weight pieces: (tile, kt0, nkt, xt chunk, xt offset)
        wm = {ch: cpool.tile([128, KTC * OPC], F16, tag=f"wm{ch}", name=f"wmt{ch}")
              for ch in range(NCHUNK)}
        vt = {ch: cpool.tile([128, KTC * OPC], I16, tag=f"vt{ch}", name=f"vtt{ch}")
              for ch in range(NCHUNK)}

        # --- DMA schedule -------------------------------------------------
        nc.sync.dma_start(out=w_sb[0][:], in_=w_dr[0][:])
        nc.scalar.dma_start(out=a2_sb[:], in_=a2p_dr[:])
        nc.scalar.dma_start(out=xt_sb[0][:], in_=xt_dr[0][:])
        nc.scalar.dma_start(out=xt_sb[1][:], in_=xt_dr[1][:])
        nc.sync.dma_start(out=w_sb[1][:], in_=w_dr[1][:])
        nc.scalar.dma_start(out=xt_sb[2][:], in_=xt_dr[2][:])
        nc.sync.dma_start(out=w_sb[2][:], in_=w_dr[2][:])
        nc.scalar.dma_start(out=xt_sb[3][:], in_=xt_dr[3][:])
        nc.sync.dma_start(out=w_sb[3][:], in_=w_dr[3][:])

        # --- main pipeline ------------------------------------------------
        HALF = OPC // 2
        psum_h = [ppool.tile([T, HALF], F32, tag=f"main{h}", name=f"psum{h}")
                  for h in range(2)]

        def unpack(wt, vtt, nkt, r):
            # vt[p, r-block, k*128+j] = W16[p, k*128+j] & (7<<3r)
            # (= code * 8^r; the 8^r rides through the matmul, divided
            # out on host).  r-major vt layout: contiguous TS writes.
            in0 = wt[:]
            out = vtt[:, r * nkt * 128:(r + 1) * nkt * 128].bitcast(I32)
            m = 7 << (3 * r)
            nc.vector.tensor_scalar(
                out, in0, (m << 16) | m, None, ALU.bitwise_and)

        def scale(eng, vtt, wmt, nkt, h):
            # wm[p, k, r*128+j] = vt[p, r, k*128+j] * a2p[p, r*128+j]
            # h None: all four r-blocks in one op; else the two of half h
            r0, nr = (0, NR) if h is None else (2 * h, 2)
            in0 = vtt[:].rearrange("p (r k j) -> p k r j", r=NR,
                                   j=2 * NWC)[:, :, r0:r0 + nr]
            out = wmt[:].rearrange("p (k r j) -> p k r j", r=NR,
                                   j=2 * NWC)[:, :, r0:r0 + nr]
            in1 = a2_sb[:, r0 * 128:(r0 + nr) * 128].rearrange(
                "p (r j) -> p r j", r=nr).unsqueeze(1).broadcast_to(
                [128, nkt, nr, 2 * NWC])
            eng.tensor_tensor(out, in0, in1, ALU.mult)

        def mms(wmt, nkt, kt0, ch, xoff, h):
            for kl in range(nkt):
                kt = kt0 + kl
                nc.tensor.matmul(
                    psum_h[h][:],
                    xt_sb[ch][:, (xoff + kl) * T:(xoff + kl + 1) * T],
                    wmt[:, kl * OPC + h * HALF:kl * OPC + (h + 1) * HALF],
                    start=(kt == 0),
                    stop=(kt == NKT - 1),
                )

        pieces = [(wm[ch], vt[ch], w_sb[ch], KTC, ch * KTC, ch, 0)
                   for ch in range(NCHUNK)]
        # h=0 consumes fields r=0,1 ; h=1 consumes r=2,3
        last = len(pieces) - 1
        for pi, (wmt, vtt, wt, nkt, kt0, ch, xoff) in enumerate(pieces):
            if pi < last:
                for r in range(NR):
                    unpack(wt, vtt, nkt, r)
                scale(nc.vector, vtt, wmt, nkt, None)
                mms(wmt, nkt, kt0, ch, xoff, 0)
                mms(wmt, nkt, kt0, ch, xoff, 1)
            else:
                for h in (1, 0):
                    unpack(wt, vtt, nkt, 2 * h)
                    unpack(wt, vtt, nkt, 2 * h + 1)
                    scale(nc.vector, vtt, wmt, nkt, h)
                    mms(wmt, nkt, kt0, ch, xoff, h)

        # --- output: two pipelined fp16 halves (independent PSUM banks) --
        out_a = opool.tile([T, HALF], F16, tag="out_a")
        out_b = opool.tile([T, HALF], F16, tag="out_b")
        nc.scalar.copy(out=out_b[:], in_=psum_h[1][:])
        nc.scalar.dma_start(out=z[:, HALF:], in_=out_b[:])
        nc.vector.tensor_copy(out_a[:], psum_h[0][:])
        nc.sync.dma_start(out=z[:, :HALF], in_=out_a[:])
    nc.finalize()
    return nc


def _get_program():
    if "nc" not in _PROGRAM_CACHE:
        _PROGRAM_CACHE["nc"] = build_program()
    return _PROGRAM_CACHE["nc"]


# ---------------------------------------------------------------- entry
def kernel(**inputs):
    from concourse.bass_utils import run_bass_kernel_spmd

    in_maps, corr = _prepare(inputs)
    nc = _get_program()
    res = run_bass_kernel_spmd(nc, in_maps, list(range(NCORES)))
    zf = np.concatenate(
        [res.results[c]["z"].astype(np.float32) for c in range(NCORES)], axis=1)
    rs = np.tile(np.repeat(8.0 ** -np.arange(NR), 2 * NWC), NCORES)
    out_reorder = np.asarray(inputs["out_reorder"], np.int64)
    y = (zf * rs[None, :] + corr)[:, out_reorder]
    return y.reshape(1, T, OUT_F).astype(np.float32)



# revision 1
# speedup vs baseline: 1.1692x; 1.1692x over previous
"""BCQLinear (3-bit binary-coded quantized linear) Trainium2 kernel.

Full-input contract: kernel(**inputs) takes the unsharded inputs of
nn_BCQLinear_88510686036218 and returns the full [1, 128, 4096] output.

Math: w = alpha*(2*S-7) + beta with S in [0,8) the 3-bit code, then
y = (x[:, in_reorder] @ w)[:, out_reorder].
Split: device computes y_q = x @ (2*alpha*S); host adds the rank-32
beta part corr = xsum.T @ (beta - 7*alpha) (0.01% of the FLOPs).

Sharding: out-features split 8 ways (512 cols/core), x replicated.

Per-core device program (SPMD, one Bass program):
  - Contraction rows are band-packed: K-tile kt in [0,32), partition p:
    dequant row i(kt,p) = 128*(p//4) + 4*kt + (p%4), so a single [128,512]
    alpha tile (a2p[p,:] ~ alpha[p//4,:]) serves every K-tile.
  - Codes arrive as packed int32 words with 4 3-bit fields per int16
    half (field r at bits [3r,3r+3)); code (j, r) maps to out-col
    o' = r*128 + j.  Dequant is two DVE passes per 8-K-tile chunk:
      unpack: vt = W32 & ((7<<3r) replicated in both halves) -- one
              single-op int32 TensorScalar per r; leaves the code
              scaled by 8^r, which rides through the matmul into z
              and is divided out on the host.
      scale:  wm = vt(int16) * a2p(fp16) -- one full-width TensorTensor
              (the last chunk splits it per column half for the tail).
  - Matmuls: stationary xt[128, T] per K-tile, streaming wm [128, 256]
    halves into two PSUM banks (fp16 operands, fp32 PSUM).
  - All DMAs ride the two HWDGE queues (sync: weight chunks, scalar:
    a2p + xt); keeping the SWDGE/gpsimd queue idle avoids descriptor-
    ring SBUF traffic that measurably slows concurrent DVE ops.
  - The last chunk runs its h1 half first so the two output halves'
    copy + DMA + HBM-receipt chains overlap.
"""
import numpy as np
from contextlib import ExitStack

import concourse.bass as bass
import concourse.mybir as mybir
import concourse.tile as tile
from concourse import bacc

IN_F, OUT_F, WBITS, GS, OFI = 4096, 4096, 3, 128, 128
NG, NB = 32, 32
NCORES = 8
OPC = OUT_F // NCORES        # 512 out-cols per core
NKT = 32                     # K-tiles of 128 rows
NR = 4                       # fields per int16 half
NWC = OPC // (2 * NR)        # 64 packed words per (partition, K-tile)
T = 128                      # tokens
NCHUNK = 4                   # w pipeline chunks
KTC = NKT // NCHUNK          # K-tiles per chunk (8)

F32 = mybir.dt.float32
F16 = mybir.dt.float16
BF16 = mybir.dt.bfloat16
I32 = mybir.dt.int32
I16 = mybir.dt.int16
ALU = mybir.AluOpType

_PROGRAM_CACHE = {}


# ---------------------------------------------------------------- host prep
def _dequant_codes(qweight):
    """S[i, o] in [0,8): w = alpha*(2S-7)+beta."""
    qw = np.asarray(qweight, dtype=np.uint32).reshape(NG, NB, WBITS, GS * OFI // 32)
    bits = (qw[..., None] >> np.arange(32, dtype=np.uint32)) & 1
    bits = bits.reshape(NG, NB, WBITS, GS, OFI)
    S = (bits * (2 ** np.arange(WBITS, dtype=np.uint32))[:, None, None]).sum(axis=2)
    return S.transpose(0, 2, 1, 3).reshape(IN_F, OUT_F).astype(np.uint32)


def _band_rows():
    kt, p = np.meshgrid(np.arange(NKT), np.arange(128), indexing="ij")
    return 128 * (p // 4) + 4 * kt + (p % 4)      # [NKT, 128]


def _prepare(inputs):
    x = np.asarray(inputs["x"], np.float32).reshape(-1, IN_F)
    alpha = np.asarray(inputs["alpha"], np.float32)
    beta = np.asarray(inputs["beta"], np.float32)
    in_reorder = np.asarray(inputs["in_reorder"], np.int64)
    xf = x[:, in_reorder]

    S = _dequant_codes(inputs["qweight"])          # [IN_F, OUT_F] uint32
    rows = _band_rows()                            # [NKT, 128]
    XT = np.ascontiguousarray(
        xf[:, rows.reshape(-1)].T.reshape(NKT, 128, T).transpose(1, 0, 2)
    ).reshape(128, NKT * T).astype(np.float16)     # [p, kt*T]

    # host-side beta part: corr[t, o] = sum_g xsum[g,t] * (beta-7a)[g,o]
    xsum = xf.reshape(T, NG, GS).sum(axis=2, dtype=np.float64)   # [T, NG]
    Bfull = beta.astype(np.float64) - 7.0 * alpha.astype(np.float64)
    corr = (xsum @ Bfull).astype(np.float32)       # [T, OUT_F]

    CW = KTC * NWC                                 # 512 words/chunk/partition
    XQ = NKT * T // NCHUNK
    in_maps = []
    for c in range(NCORES):
        cols = slice(OPC * c, OPC * (c + 1))
        # codes for this core in banded row order: [p, kt, o']
        Sc = S[rows.reshape(-1), cols].reshape(NKT, 128, OPC).transpose(1, 0, 2)
        # pack fields: o' = r*128 + 2c' + h  ->  bits [3r+16h, +3)
        W = np.zeros((128, NKT, NWC), np.uint32)
        for r in range(NR):
            for h in range(2):
                W |= Sc[:, :, r * 128 + h::2][:, :, :NWC] << (3 * r + 16 * h)
        W = W.reshape(128, NKT * NWC)
        a2p = (2.0 * alpha[np.arange(128) // 4][:, cols]).astype(np.float16)
        im = dict(a2p=a2p)
        for ch in range(NCHUNK):
            im[f"w{ch}"] = np.ascontiguousarray(
                W[:, ch * CW:(ch + 1) * CW]).view(np.int32)
        for ch in range(NCHUNK):
            im[f"xt{ch}"] = np.ascontiguousarray(XT[:, ch * XQ:(ch + 1) * XQ])
        in_maps.append(im)
    return in_maps, corr


# ---------------------------------------------------------------- program
def build_program():
    nc = bacc.Bacc("TRN2")
    CW = KTC * NWC            # packed words per chunk per partition (512)
    HW2 = CW // 2             # 256 words per chunk-0 half
    XQ = NKT * T // NCHUNK    # xt columns per chunk (1024)

    w_dr = {ch: nc.declare_dram_parameter(f"w{ch}", [128, CW], I32,
                                          isOutput=False)
            for ch in range(NCHUNK)}
    xt_dr = [nc.declare_dram_parameter(f"xt{ch}", [128, XQ], F16,
                                       isOutput=False)
             for ch in range(NCHUNK)]
    a2p_dr = nc.declare_dram_parameter("a2p", [128, OPC], F16, isOutput=False)
    z = nc.declare_dram_parameter("z", [T, OPC], F16, isOutput=True)

    with tile.TileContext(nc) as tc, ExitStack() as ctx:
        cpool = ctx.enter_context(tc.tile_pool(name="const", bufs=1))
        opool = ctx.enter_context(tc.tile_pool(name="out", bufs=1))
        ppool = ctx.enter_context(tc.tile_pool(name="psum", bufs=1, space="PSUM"))

        # --- static tiles --------------------------------------------------
        w_sb = {ch: cpool.tile([128, CW], I32, tag=f"w{ch}", name=f"wsb{ch}")
                for ch in range(NCHUNK)}
        xt_sb = [cpool.tile([128, XQ], F16, tag=f"xt{ch}", name=f"xtsb{ch}")
                 for ch in range(NCHUNK)]
        a2_sb = cpool.tile([128, OPC], F16, tag="a2")
        # dequantized weight pieces: (tile, kt0, nkt, xt chunk, xt offset)
        wm = {ch: cpool.tile([128, KTC * OPC], F16, tag=f"wm{ch}", name=f"wmt{ch}")
              for ch in range(NCHUNK)}
        vt = {ch: cpool.tile([128, KTC * OPC], I16, tag=f"vt{ch}", name=f"vtt{ch}")
              for ch in range(NCHUNK)}

        # --- DMA schedule -------------------------------------------------
        nc.sync.dma_start(out=w_sb[0][:], in_=w_dr[0][:])
        nc.scalar.dma_start(out=a2_sb[:], in_=a2p_dr[:])
        nc.scalar.dma_start(out=xt_sb[0][:], in_=xt_dr[0][:])
        nc.scalar.dma_start(out=xt_sb[1][:], in_=xt_dr[1][:])
        nc.sync.dma_start(out=w_sb[1][:], in_=w_dr[1][:])
        nc.scalar.dma_start(out=xt_sb[2][:], in_=xt_dr[2][:])
        nc.sync.dma_start(out=w_sb[2][:], in_=w_dr[2][:])
        nc.scalar.dma_start(out=xt_sb[3][:], in_=xt_dr[3][:])
        nc.sync.dma_start(out=w_sb[3][:], in_=w_dr[3][:])

        # --- main pipeline ------------------------------------------------
        HALF = OPC // 2
        psum_h = [ppool.tile([T, HALF], F32, tag=f"main{h}", name=f"psum{h}")
                  for h in range(2)]

        def unpack(wt, vtt, nkt, r):
            # vt[p, r-block, k*128+j] = W16[p, k*128+j] & (7<<3r)
            # (= code * 8^r; the 8^r rides through the matmul, divided
            # out on host).  r-major vt layout: contiguous TS writes.
            in0 = wt[:]
            out = vtt[:, r * nkt * 128:(r + 1) * nkt * 128].bitcast(I32)
            m = 7 << (3 * r)
            nc.vector.tensor_scalar(
                out, in0, (m << 16) | m, None, ALU.bitwise_and)

        def scale(eng, vtt, wmt, nkt, h):
            # wm[p, k, r*128+j] = vt[p, r, k*128+j] * a2p[p, r*128+j]
            # h None: all four r-blocks in one op; else the two of half h
            r0, nr = (0, NR) if h is None else (2 * h, 2)
            in0 = vtt[:].rearrange("p (r k j) -> p k r j", r=NR,
                                   j=2 * NWC)[:, :, r0:r0 + nr]
            out = wmt[:].rearrange("p (k r j) -> p k r j", r=NR,
                                   j=2 * NWC)[:, :, r0:r0 + nr]
            in1 = a2_sb[:, r0 * 128:(r0 + nr) * 128].rearrange(
                "p (r j) -> p r j", r=nr).unsqueeze(1).broadcast_to(
                [128, nkt, nr, 2 * NWC])
            eng.tensor_tensor(out, in0, in1, ALU.mult)

        def mms(wmt, nkt, kt0, ch, xoff, h):
            for kl in range(nkt):
                kt = kt0 + kl
                nc.tensor.matmul(
                    psum_h[h][:],
                    xt_sb[ch][:, (xoff + kl) * T:(xoff + kl + 1) * T],
                    wmt[:, kl * OPC + h * HALF:kl * OPC + (h + 1) * HALF],
                    start=(kt == 0),
                    stop=(kt == NKT - 1),
                )

        pieces = [(wm[ch], vt[ch], w_sb[ch], KTC, ch * KTC, ch, 0)
                   for ch in range(NCHUNK)]
        # h=0 consumes fields r=0,1 ; h=1 consumes r=2,3
        last = len(pieces) - 1
        for pi, (wmt, vtt, wt, nkt, kt0, ch, xoff) in enumerate(pieces):
            if pi < last:
                for r in range(NR):
                    unpack(wt, vtt, nkt, r)
                scale(nc.vector, vtt, wmt, nkt, None)
                mms(wmt, nkt, kt0, ch, xoff, 0)
                mms(wmt, nkt, kt0, ch, xoff, 1)
            else:
                for h in (1, 0):
                    unpack(wt, vtt, nkt, 2 * h)
                    unpack(wt, vtt, nkt, 2 * h + 1)
                    scale(nc.vector, vtt, wmt, nkt, h)
                    mms(wmt, nkt, kt0, ch, xoff, h)

        # --- output: two pipelined fp16 halves (independent PSUM banks) --
        out_a = opool.tile([T, HALF], F16, tag="out_a")
        out_b = opool.tile([T, HALF], F16, tag="out_b")
        nc.scalar.copy(out=out_b[:], in_=psum_h[1][:])
        nc.scalar.dma_start(out=z[:, HALF:], in_=out_b[:])
        nc.vector.tensor_copy(out_a[:], psum_h[0][:])
        nc.sync.dma_start(out=z[:, :HALF], in_=out_a[:])
    nc.finalize()
    return nc


def _get_program():
    if "nc" not in _PROGRAM_CACHE:
        _PROGRAM_CACHE["nc"] = build_program()
    return _PROGRAM_CACHE["nc"]


# ---------------------------------------------------------------- entry
def kernel(**inputs):
    from concourse.bass_utils import run_bass_kernel_spmd

    in_maps, corr = _prepare(inputs)
    nc = _get_program()
    res = run_bass_kernel_spmd(nc, in_maps, list(range(NCORES)))
    zf = np.concatenate(
        [res.results[c]["z"].astype(np.float32) for c in range(NCORES)], axis=1)
    rs = np.tile(np.repeat(8.0 ** -np.arange(NR), 2 * NWC), NCORES)
    out_reorder = np.asarray(inputs["out_reorder"], np.int64)
    y = (zf * rs[None, :] + corr)[:, out_reorder]
    return y.reshape(1, T, OUT_F).astype(np.float32)

